# revision 7
# baseline (speedup 1.0000x reference)
"""Trainium2 Bass kernel for nn_Attention_512 (ragged per-group attention scorer).

Math (per group g, n = lengths[g], using only the first n positions):
    Q = info @ Wq ; K = info @ Wk ; scores = Q K^T  (keys masked to n)
    attn = softmax(scores) ; ctx = attn @ (info @ Wv)
    w = (((ctx W1 + b1) W2 + b2) W3 + b3) W4 + b4        # all linear!
    out[:, g] = raw[g] @ (w * mask)   (+ length==1 onehot special case)

Algebraic folds used (all linear, validated to ~6e-6 rel err vs reference):
    A   = Wq @ Wk^T                  -> scores = info A info^T  (saves Q or K)
    vWc = Wv @ W1 @ W2 @ W3 @ W4     -> per-key scalar v-values  [F]
    c   = ((b1 W2 + b2) W3 + b3) W4 + b4  (scalar)
    w[l] = (E[l,:] @ vs) / Z[l] + c,  E = exp(scores - max), vs = info @ vWc

Distribution: 128 groups sorted by length desc; rank 8j+c -> core c slot j.
All 8 cores run ONE SPMD graph with per-slot bucket lengths B[j] =
max length in rank slice [8j, 8j+8)  (~6% padding).  Padded key columns are
killed by a -1e30 bias folded into the scores matmul as an extra K=1 row.
Slots are packed in PAIRS along the matmul N axis so the f32r score-path
matmuls hit the N>=256 full-rate (1 cyc/col) regime.

dtypes: score path f32r (~13 mantissa bits), everything else bf16,
accumulation fp32.  Expected rel err vs fp32 reference ~3e-3.
"""
import math
import numpy as np
import ml_dtypes

import concourse.bass as bass
import concourse.tile as tile
from concourse import bacc, mybir
from concourse.bass_utils import run_bass_kernel_spmd
from concourse.masks import make_identity

G, S, L, F = 128, 2048, 256, 512
N_CORES = 8
SLOTS = G // N_CORES  # 16
NEG = -1.0e30


def _build_graph(B, pair_of, buf_order, offs, total_w, c_const):
    """Build the single SPMD graph.

    B: per-slot bucket lengths [16] (slot index = output row index)
    buf_order: slot ids in buffer order (pair members adjacent)
    offs: per-slot column offset in the packed buffers
    pair_of: list of 8 (slot_a, slot_b) tuples (buffer-adjacent)
    total_w: sum of B
    """
    f32 = mybir.dt.float32
    f32r = mybir.dt.float32r
    bf16 = mybir.dt.bfloat16
    KC = 4  # f-chunks (F=512)

    nc = bacc.Bacc("TRN2", target_bir_lowering=False, debug=False,
                   num_devices=N_CORES)
    A_d = nc.dram_tensor("A", [F, F], f32r, kind="ExternalInput").ap()
    vWc_d = nc.dram_tensor("vWcrep", [128, KC, 128], f32r, kind="ExternalInput").ap()
    ones_d = nc.dram_tensor("onesr", [1, 128], f32r, kind="ExternalInput").ap()
    info_d = nc.dram_tensor("infoTp", [F, total_w], f32r, kind="ExternalInput").ap()
    mask_d = nc.dram_tensor("maskf", [1, total_w], f32r, kind="ExternalInput").ap()
    raw_d = nc.dram_tensor("rawTp", [total_w, S], bf16, kind="ExternalInput").ap()
    out_d = nc.dram_tensor("out", [SLOTS, S], f32, kind="ExternalOutput").ap()

    with tile.TileContext(nc) as tc:
        with tc.tile_pool(name="const", bufs=1) as const_p, \
             tc.tile_pool(name="info", bufs=1) as info_p, \
             tc.tile_pool(name="ptsb", bufs=2) as ptsb_p, \
             tc.tile_pool(name="esb", bufs=3) as e_p, \
             tc.tile_pool(name="etsb", bufs=3) as et_p, \
             tc.tile_pool(name="rawsb", bufs=3) as raw_p, \
             tc.tile_pool(name="vecs", bufs=4) as vec_p, \
             tc.tile_pool(name="pt_ps", bufs=2, space="PSUM") as ptps_p, \
             tc.tile_pool(name="sc_ps", bufs=2, space="PSUM") as scps_p, \
             tc.tile_pool(name="misc_ps", bufs=2, space="PSUM") as miscps_p, \
             tc.tile_pool(name="out_ps", bufs=2, space="PSUM") as outps_p:

            # ---- resident constants ----
            A_sb = const_p.tile([128, KC, F], f32r)      # A rows f'-chunked
            nc.sync.dma_start(out=A_sb, in_=A_d.rearrange("(k p) f -> p k f", p=128))
            vWc_sb = const_p.tile([128, KC, 128], f32r)
            nc.sync.dma_start(out=vWc_sb, in_=vWc_d)
            mask_sb = const_p.tile([1, total_w], f32r)
            nc.sync.dma_start(out=mask_sb, in_=mask_d)
            info_sb = info_p.tile([128, KC, total_w], f32r)
            nc.sync.dma_start(out=info_sb,
                              in_=info_d.rearrange("(k p) w -> p k w", p=128))
            ident = const_p.tile([128, 128], bf16)
            make_identity(nc, ident[:])
            ones_r = const_p.tile([1, 128], f32r)
            nc.sync.dma_start(out=ones_r, in_=ones_d)
            ones_b = const_p.tile([128, 1], bf16)
            nc.vector.memset(ones_b, 1.0)

            for (sa, sb_) in pair_of:
                poff = offs[sa]
                W = B[sa] + B[sb_]
                # ---- PT = A^T @ infoT for the pair ----
                pt_sb = ptsb_p.tile([128, KC, 640], f32r, tag="ptsb")
                for m in range(KC):
                    pt_ps = ptps_p.tile([128, 512], f32, tag="ptps")
                    for k in range(KC):
                        nc.tensor.matmul(pt_ps[:, :W],
                                         A_sb[:, k, m * 128:(m + 1) * 128],
                                         info_sb[:, k, poff:poff + W],
                                         start=(k == 0), stop=(k == KC - 1))
                    nc.scalar.copy(out=pt_sb[:, m, :W], in_=pt_ps[:, :W])
                # ---- vs row for the pair:  vs = vWc^T @ infoT ----
                vs_ps = miscps_p.tile([128, 512], f32, tag="misc")
                for k in range(KC):
                    nc.tensor.matmul(vs_ps[:, :W], vWc_sb[:, k, :],
                                     info_sb[:, k, poff:poff + W],
                                     start=(k == 0), stop=(k == KC - 1))
                vsrow = vec_p.tile([1, 512], bf16, tag="vsrow")
                nc.vector.tensor_copy(out=vsrow[:, :W], in_=vs_ps[0:1, :W])

                for slot in (sa, sb_):
                    n = B[slot]
                    soff = offs[slot]
                    own = soff - poff
                    kl = (n + 127) // 128
                    e_tiles = []
                    # ---- scores + softmax-exp per l-chunk ----
                    for lc in range(kl):
                        sz = min(128, n - 128 * lc)
                        sc_ps = scps_p.tile([128, 512], f32, tag="scps")
                        for k in range(KC):
                            nc.tensor.matmul(
                                sc_ps[:, :W],
                                pt_sb[:, k, own + 128 * lc: own + 128 * lc + 128],
                                info_sb[:, k, poff:poff + W],
                                start=(k == 0), stop=False)
                        # padded-key mask as rank-1 update over the pair width
                        nc.tensor.matmul(sc_ps[:, :W], ones_r[0:1, :],
                                         mask_sb[:, poff:poff + W],
                                         start=False, stop=True)
                        nmx = vec_p.tile([128, 1], f32, tag="nmx")
                        nc.vector.tensor_reduce(
                            out=nmx[:sz], in_=sc_ps[:sz, own:own + n],
                            op=mybir.AluOpType.max, axis=mybir.AxisListType.X,
                            negate=True)
                        e_t = e_p.tile([128, 256], bf16, tag="E")
                        nc.scalar.activation(
                            out=e_t[:sz, :n], in_=sc_ps[:sz, own:own + n],
                            func=mybir.ActivationFunctionType.Exp,
                            bias=nmx[:sz], scale=1.0)
                        e_tiles.append(e_t)
                    # ---- ET = E^T ----
                    et_tiles = []
                    for mc in range(kl):
                        szm = min(128, n - 128 * mc)
                        et_t = et_p.tile([128, 256], bf16, tag="ET")
                        for lc in range(kl):
                            szl = min(128, n - 128 * lc)
                            tp_ps = miscps_p.tile([128, 128], bf16, tag="misc")
                            nc.tensor.transpose(
                                tp_ps[:szm, :szl],
                                e_tiles[lc][:szl, 128 * mc:128 * mc + szm],
                                ident[:szl, :szl])
                            nc.vector.tensor_copy(
                                out=et_t[:szm, 128 * lc:128 * lc + szl],
                                in_=tp_ps[:szm, :szl])
                        et_tiles.append(et_t)
                    # ---- vs columns and u/Z rows ----
                    vso = []
                    for mc in range(kl):
                        szm = min(128, n - 128 * mc)
                        vt_ps = miscps_p.tile([128, 1], bf16, tag="misc")
                        nc.tensor.transpose(
                            vt_ps[:szm, 0:1],
                            vsrow[0:1, own + 128 * mc: own + 128 * mc + szm],
                            ident[0:1, 0:1])
                        vo = vec_p.tile([128, 1], bf16, tag="vso")
                        nc.vector.tensor_copy(out=vo[:szm, 0:1], in_=vt_ps[:szm, 0:1])
                        vso.append(vo)
                    u_ps = miscps_p.tile([1, 512], f32, tag="misc")
                    z_ps = miscps_p.tile([1, 512], f32, tag="misc")
                    for mc in range(kl):
                        szm = min(128, n - 128 * mc)
                        nc.tensor.matmul(u_ps[:1, :n], vso[mc][:szm, 0:1],
                                         et_tiles[mc][:szm, :n],
                                         start=(mc == 0), stop=(mc == kl - 1))
                        nc.tensor.matmul(z_ps[:1, :n], ones_b[:szm, 0:1],
                                         et_tiles[mc][:szm, :n],
                                         start=(mc == 0), stop=(mc == kl - 1))
                    # ---- w = u/Z + c  (bf16 row) ----
                    rz = vec_p.tile([1, 512], f32, tag="rz")
                    nc.vector.reciprocal(out=rz[:, :n], in_=z_ps[0:1, :n])
                    wtmp = vec_p.tile([1, 512], f32, tag="wtmp")
                    nc.vector.tensor_mul(out=wtmp[:, :n], in0=u_ps[0:1, :n],
                                         in1=rz[:, :n])
                    wrow = vec_p.tile([1, 512], bf16, tag="wrow")
                    nc.scalar.activation(out=wrow[:, :n], in_=wtmp[:, :n],
                                         func=mybir.ActivationFunctionType.Copy,
                                         bias=float(c_const), scale=1.0)
                    # ---- w columns ----
                    wcols = []
                    for lc in range(kl):
                        sz = min(128, n - 128 * lc)
                        wt_ps = miscps_p.tile([128, 1], bf16, tag="misc")
                        nc.tensor.transpose(
                            wt_ps[:sz, 0:1],
                            wrow[0:1, 128 * lc:128 * lc + sz],
                            ident[0:1, 0:1])
                        wc = vec_p.tile([128, 1], bf16, tag="wcol")
                        nc.vector.tensor_copy(out=wc[:sz, 0:1], in_=wt_ps[:sz, 0:1])
                        wcols.append(wc)
                    # ---- final: out[slot] = w^T @ rawT ----
                    r_tiles = []
                    for lc in range(kl):
                        sz = min(128, n - 128 * lc)
                        rt = raw_p.tile([128, S], bf16, tag="raw")
                        nc.sync.dma_start(
                            out=rt[:sz, :],
                            in_=raw_d[soff + 128 * lc: soff + 128 * lc + sz, :])
                        r_tiles.append(rt)
                    for j in range(S // 512):
                        o_ps = outps_p.tile([1, 512], f32, tag="ops")
                        for lc in range(kl):
                            sz = min(128, n - 128 * lc)
                            nc.tensor.matmul(o_ps[:, :],
                                             wcols[lc][:sz, 0:1],
                                             r_tiles[lc][:sz, j * 512:(j + 1) * 512],
                                             start=(lc == 0), stop=(lc == kl - 1))
                        o_sb = vec_p.tile([1, 512], f32, tag="orow")
                        if j % 2 == 0:
                            nc.vector.tensor_copy(out=o_sb[:, :], in_=o_ps[:, :])
                        else:
                            nc.scalar.copy(out=o_sb[:, :], in_=o_ps[:, :])
                        nc.sync.dma_start(
                            out=out_d[slot:slot + 1, j * 512:(j + 1) * 512],
                            in_=o_sb[:, :])
    nc.compile()
    return nc


def _prep(inputs):
    """Host-side: fold weights, sort groups, pack per-core padded buffers."""
    raw = np.asarray(inputs["raw"], np.float32)
    info = np.asarray(inputs["info"], np.float32)
    Wq = np.asarray(inputs["Wq"], np.float64)
    Wk = np.asarray(inputs["Wk"], np.float64)
    Wv = np.asarray(inputs["Wv"], np.float64)
    W1 = np.asarray(inputs["W1"], np.float64)
    b1 = np.asarray(inputs["b1"], np.float64)
    W2 = np.asarray(inputs["W2"], np.float64)
    b2 = np.asarray(inputs["b2"], np.float64)
    W3 = np.asarray(inputs["W3"], np.float64)
    b3 = np.asarray(inputs["b3"], np.float64)
    W4 = np.asarray(inputs["W4"], np.float64)
    b4 = np.asarray(inputs["b4"], np.float64)
    lengths = np.asarray(inputs["lengths"]).astype(np.int64)

    A = (Wq @ Wk.T).astype(np.float32)                      # [F, F]
    vWc = (Wv @ W1 @ W2 @ W3 @ W4)[:, 0].astype(np.float32)  # [F]
    c_const = float((((b1 @ W2 + b2) @ W3 + b3) @ W4 + b4)[0])

    order = np.argsort(-lengths, kind="stable")              # rank -> group
    # even-rounded buckets (f32r matmul N must be even)
    B = [min(L, int(lengths[order[8 * j]]) + (int(lengths[order[8 * j]]) & 1))
         for j in range(SLOTS)]
    # buffer order: pair slot j with slot 15-j, members adjacent
    buf_order = []
    pair_of = []
    for p in range(SLOTS // 2):
        buf_order += [p, SLOTS - 1 - p]
        pair_of.append((p, SLOTS - 1 - p))
    offs = {}
    off = 0
    for s in buf_order:
        offs[s] = off
        off += B[s]
    total_w = off

    vwc_rep = np.ascontiguousarray(
        np.broadcast_to(vWc.reshape(4, 128).T[:, :, None], (128, 4, 128))).astype(np.float32)
    in_maps = []
    infoT = info.transpose(0, 2, 1)                          # [G, F, L] views
    for cidx in range(N_CORES):
        infoTp = np.zeros((F, total_w), np.float32)
        rawTp = np.zeros((total_w, S), ml_dtypes.bfloat16)
        maskf = np.full((1, total_w), NEG, np.float32)
        for j in range(SLOTS):
            g = int(order[8 * j + cidx])
            n = int(lengths[g])
            o = offs[j]
            infoTp[:, o:o + n] = infoT[g, :, :n]
            rawTp[o:o + n, :] = raw[g, :, :n].T.astype(ml_dtypes.bfloat16)
            maskf[0, o:o + n] = 0.0
        in_maps.append({
            "A": A,
            "vWcrep": vwc_rep,
            "onesr": np.ones((1, 128), np.float32),
            "infoTp": infoTp, "maskf": maskf, "rawTp": rawTp,
        })
    return (in_maps, order, lengths, raw,
            dict(B=B, pair_of=pair_of, buf_order=buf_order, offs=offs,
                 total_w=total_w, c_const=c_const))


def run(inputs, trace=False):
    in_maps, order, lengths, raw, g = _prep(inputs)
    nc = _build_graph(g["B"], g["pair_of"], g["buf_order"], g["offs"],
                      g["total_w"], g["c_const"])
    res = run_bass_kernel_spmd(nc, in_maps, core_ids=list(range(N_CORES)),
                               trace=trace)
    out = np.zeros((S, G), np.float32)
    for cidx in range(N_CORES):
        o_c = res.results[cidx]["out"]                       # [16, 2048]
        for j in range(SLOTS):
            out[:, int(order[8 * j + cidx])] = o_c[j]
    for gi in np.nonzero(lengths == 1)[0]:                   # onehot special case
        out[:, gi] = raw[gi, :, 0]
    return out, res.exec_time_ns


def kernel(**inputs) -> np.ndarray:
    out, _ = run(inputs, trace=False)
    return out
